# revision 28
# baseline (speedup 1.0000x reference)
"""Trainium2 Bass kernel: multi-head attention (dense transformer block).

Computation (per batch b):
    Q = x @ Wq + bq ; K = x @ Wk + bk ; V = x @ Wv + bv        (per head)
    P = exp((Q @ K^T) / sqrt(Dh))                               (no max-sub; scores are O(1))
    out = sum_h (P @ V / rowsum(P)) @ Wd[h] + bd

Sharding: 8 cores; core c handles batch b = c // 4 and 4 heads h0 = 4*(c%4).
Each core returns a partial [L, D] output; host sums groups of 4 cores + bd.

Per-core dataflow (SPMD program, all cores identical):
  - x^T built on-chip via PE transposes (f32 data, fp32r matmul path).
  - Heads processed in 2 pairs; QKV projections (fp32r) computed as Q^T/K^T
    [128 = 2 heads stacked on partitions, L], drained to bf16, so scores
    S^T = K^T.T @ Q^T (bf16, fast-weight-load) land with the softmax axis on
    PSUM partitions; exp on ScalarE (scale fused, 1024-wide) -> P^T bf16.
  - attend: O^T = [V_h | ones*64].T @ P^T — the 64 replicated ones columns
    make PSUM rows 64..127 the softmax denominator, already broadcast, for
    free; normalize = recip + mul on DVE, no cross-partition ops needed.
  - out-proj: Y = O^T.T @ Wd accumulated over head pairs in PSUM (fp32r).
"""

import os
import sys
from contextlib import ExitStack

import numpy as np

for _p in ("/opt/trn_rl_repo", "/root/.axon_site/_ro/trn_rl_repo"):
    if os.path.isdir(_p) and _p not in sys.path:
        sys.path.append(_p)

import concourse.bass as bass
import concourse.tile as tile
from concourse import bacc, mybir
from concourse.bass import ds, ts
from concourse.bass_utils import run_bass_kernel_spmd
from concourse.masks import make_identity
from concourse.tile_rust import add_dep_helper

F32 = mybir.dt.float32
F32R = mybir.dt.float32r
BF16 = mybir.dt.bfloat16

# Problem sizes (hardcoded per contract).
DMODEL, HEADS, DHEAD = 1024, 16, 64
B, L = 2, 2048
NCORES = 8
H_PER_CORE = B * HEADS // NCORES          # 4 heads per core
NPAIR = H_PER_CORE // 2                   # head pairs per core
P = 128                                   # partitions
KT = DMODEL // P                          # 8 k-tiles over dmodel
NLT = L // P                              # 16 l-tiles
LCH = 512                                 # matmul free-dim chunk (one psum bank)
ECH = 512                                 # exp chunk
NEC = L // ECH                            # 2 exp chunks
MCH = 512                                 # m-chunk for out-proj
NMC = DMODEL // MCH

BLOCK_DEPS = os.environ.get("ATT_BLOCK_DEPS", "1") == "1"


def build_nc():
    """Build the SPMD Bass program for one core."""
    nc = bacc.Bacc("TRN2", target_bir_lowering=False, debug=False,
                   num_devices=NCORES)

    x_d = nc.dram_tensor("x", [L, DMODEL], F32, kind="ExternalInput").ap()
    wq_d = nc.dram_tensor("wq", [DMODEL, H_PER_CORE * DHEAD], F32, kind="ExternalInput").ap()
    wk_d = nc.dram_tensor("wk", [DMODEL, H_PER_CORE * DHEAD], F32, kind="ExternalInput").ap()
    wv_d = nc.dram_tensor("wv", [DMODEL, H_PER_CORE * DHEAD], F32, kind="ExternalInput").ap()
    wd_d = nc.dram_tensor("wd", [H_PER_CORE * DHEAD, DMODEL], F32, kind="ExternalInput").ap()
    bq_d = nc.dram_tensor("bq", [H_PER_CORE * DHEAD], F32, kind="ExternalInput").ap()
    bk_d = nc.dram_tensor("bk", [H_PER_CORE * DHEAD], F32, kind="ExternalInput").ap()
    bv_d = nc.dram_tensor("bv", [H_PER_CORE * DHEAD], F32, kind="ExternalInput").ap()
    y_d = nc.dram_tensor("y", [L, DMODEL], F32, kind="ExternalOutput").ap()

    with ExitStack() as ctx:
        tc = ctx.enter_context(tile.TileContext(nc))
        _body(nc, tc, ctx, x_d, wq_d, wk_d, wv_d, wd_d, bq_d, bk_d, bv_d, y_d)
    nc.compile()
    return nc


def _body(nc, tc, ctx, x_d, wq_d, wk_d, wv_d, wd_d, bq_d, bk_d, bv_d, y_d):
    const = ctx.enter_context(tc.tile_pool(name="const", bufs=1))
    sb = ctx.enter_context(tc.tile_pool(name="sb", bufs=1))
    psum = ctx.enter_context(tc.tile_pool(name="psum", bufs=1, space="PSUM"))

    ident = const.tile([P, P], F32)
    make_identity(nc, ident)

    # biases: [pair*128 + i] layout matches head-pair partition stacking.
    bias_sb = const.tile([P, 3, NPAIR], F32)
    for i, b_d in enumerate((bq_d, bk_d, bv_d)):
        for p in range(NPAIR):
            nc.sync.dma_start(bias_sb[:, i, p:p + 1],
                              b_d.rearrange("(a p) -> a p", p=P)[p:p + 1, :]
                              .rearrange("a p -> p a"))
    # bv replicated across partitions (free-axis bias for the V drain)
    bv_rep = const.tile([P, NPAIR * P], F32)
    nc.sync.dma_start(bv_rep, bass.AP(tensor=bv_d.tensor, offset=0,
                                      ap=[[0, P], [1, NPAIR * P]]))

    # shared [128,1024] psum slots for scores & transposes
    def sc_tile(shape=None, dt=F32):
        return psum.tile(shape or [P, ECH], dt, tag="sctr", bufs=4, name="sctr")

    # qkv weights for all pairs first (so QKV can start asap), bf16 on device
    w_sb = const.tile([P, NPAIR, 3, KT, P], BF16)
    for p in range(NPAIR):
        ws = sb.tile([P, 3, KT, P], F32, tag="wstage", bufs=1)
        for i, w_d in enumerate((wq_d, wk_d, wv_d)):
            nc.sync.dma_start(
                ws[:, i],
                w_d.rearrange("(kt k) m -> k kt m", k=P)[:, :, ds(p * P, P)])
        nc.vector.tensor_copy(w_sb[:, p], ws)

    # out-proj weights, bf16 on device
    wd_sb = const.tile([P, NPAIR, DMODEL], BF16)
    wds = sb.tile([P, NPAIR, DMODEL], F32, tag="wstage2", bufs=1)
    nc.sync.dma_start(wds, wd_d.rearrange("(pp k) m -> k pp m", k=P))
    nc.vector.tensor_copy(wd_sb, wds)

    # ---- phase 0: x^T bf16 via PE transposes (f32 in, bf16 out drain) ----
    xt = sb.tile([P, KT, L], BF16)
    for lt in range(NLT):
        xs = sb.tile([P, DMODEL], F32, tag="xstage", bufs=4)
        eng = nc.sync if lt % 2 == 0 else nc.gpsimd
        eng.dma_start(xs, x_d[ds(lt * P, P), :])
        for kt in range(KT):
            tp = sc_tile([P, P], F32)
            nc.tensor.transpose(tp, xs[:, ds(kt * P, P)], ident)
            nc.vector.tensor_copy(xt[:, kt, ds(lt * P, P)], tp)

    o_norm = sb.tile([P, NPAIR, L], BF16)

    # ---- per pair: K -> Q(chunk0) -> V -> blocks(chunk0) -> Q(chunk1)... ----
    def qkv_proj(dst, p, i, lcs):
        for lc in lcs:
            ps = psum.tile([P, LCH], F32, tag="qkvp", bufs=2, name="qkvps")
            for kt in range(KT):
                nc.tensor.matmul(
                    ps, lhsT=w_sb[:, p, i, kt],
                    rhs=xt[:, kt, ds(lc * LCH, LCH)],
                    start=(kt == 0), stop=(kt == KT - 1))
            nc.vector.tensor_scalar_add(
                dst[:, ds(lc * LCH, LCH)], ps, bias_sb[:, i, p:p + 1])

    for p in range(NPAIR):
        qT = sb.tile([P, L], BF16, tag="qkv0", bufs=NPAIR)
        kT_sb = sb.tile([P, L], BF16, tag="qkv1", bufs=NPAIR)
        qkv_proj(kT_sb, p, 1, range(L // LCH))
        qkv_proj(qT, p, 0, range(ECH // LCH))

        # V computed directly in [l', d] layout: per head [V_h (64) | ones]
        vt = sb.tile([P, NLT, 2 * P], BF16, tag="vt", bufs=NPAIR)
        nc.vector.memset(vt[:, :, DHEAD:P], 1.0)
        nc.vector.memset(vt[:, :, P + DHEAD:2 * P], 1.0)
        for lt in range(NLT):
            vp = psum.tile([P, P], F32, tag="qkvp", bufs=2, name="vp")
            for kt in range(KT):
                nc.tensor.matmul(
                    vp, lhsT=xt[:, kt, ds(lt * P, P)],
                    rhs=w_sb[:, p, 2, kt],
                    start=(kt == 0), stop=(kt == KT - 1))
            nc.vector.tensor_add(vt[:, lt, 0:DHEAD], vp[:, 0:DHEAD],
                                 bv_rep[:, ds(p * P, DHEAD)])
            nc.vector.tensor_add(vt[:, lt, P:P + DHEAD], vp[:, DHEAD:P],
                                 bv_rep[:, ds(p * P + DHEAD, DHEAD)])

        # blocks: h-interleaved scores (dual 64-row PE tiles) -> exp -> attend
        for ec in range(NEC):
            if ec > 0:
                qkv_proj(qT, p, 0, range(ec * ECH // LCH, (ec + 1) * ECH // LCH))
            pt_tiles = [[None] * NLT, [None] * NLT]
            for lt in range(NLT):
                for h in range(2):
                    sp = sc_tile()
                    for sub in range(ECH // LCH):
                        nc.tensor.matmul(
                            sp[:, ds(sub * LCH, LCH)],
                            lhsT=kT_sb[ds(64 * h, 64), ds(lt * P, P)],
                            rhs=qT[ds(64 * h, 64),
                                   ds(ec * ECH + sub * LCH, LCH)],
                            start=True, stop=True)
                    pt = sb.tile([P, ECH], BF16, tag="pt", bufs=36)
                    nc.scalar.activation(
                        pt, sp, func=mybir.ActivationFunctionType.Exp,
                        scale=1.0 / np.sqrt(DHEAD))
                    pt_tiles[h][lt] = pt

            for sub in range(ECH // LCH):
                lc = ec * ECH + sub * LCH
                for h in range(2):
                    op = psum.tile([P, LCH], F32, tag="op", bufs=2)
                    for lt in range(NLT):
                        nc.tensor.matmul(
                            op, lhsT=vt[:, lt, ds(P * h, P)],
                            rhs=pt_tiles[h][lt][:, ds(sub * LCH, LCH)],
                            start=(lt == 0), stop=(lt == NLT - 1))
                    # rows 64..127 are the denominator, already broadcast
                    rs = sb.tile([DHEAD, LCH], F32, tag="rs", bufs=2)
                    nc.vector.reciprocal(rs, op[DHEAD:P, :])
                    nc.vector.tensor_mul(
                        o_norm[ds(64 * h, 64), p, ds(lc, LCH)],
                        op[0:DHEAD, :], rs)

            # out-projection for this chunk once the last pair finished it
            if p == NPAIR - 1:
                for lt in range(ec * NLT // NEC, (ec + 1) * NLT // NEC):
                    for mc in range(NMC):
                        yp = psum.tile([P, MCH], F32, tag="qkvp", bufs=2)
                        for pp in range(NPAIR):
                            nc.tensor.matmul(
                                yp, lhsT=o_norm[:, pp, ds(lt * P, P)],
                                rhs=wd_sb[:, pp, ds(mc * MCH, MCH)],
                                start=(pp == 0), stop=(pp == NPAIR - 1))
                        ys = sb.tile([P, MCH], F32, tag="ys", bufs=3)
                        nc.vector.tensor_copy(ys, yp)
                        nc.sync.dma_start(
                            y_d[ds(lt * P, P), ds(mc * MCH, MCH)], ys)


_NC_CACHE = {}


def _get_nc():
    if "nc" not in _NC_CACHE:
        _NC_CACHE["nc"] = build_nc()
    return _NC_CACHE["nc"]


def shard_inputs(x, Wq, bq, Wk, bk, Wv, bv, Wd, bd):
    """Build the 8 per-core input maps."""
    in_maps = []
    for c in range(NCORES):
        b = c // (NCORES // B)
        h0 = (c % (NCORES // B)) * H_PER_CORE
        hs = slice(h0, h0 + H_PER_CORE)
        in_maps.append({
            "x": np.ascontiguousarray(np.asarray(x[b], np.float32)),
            "wq": np.ascontiguousarray(np.asarray(Wq[:, hs, :], np.float32).reshape(DMODEL, -1)),
            "wk": np.ascontiguousarray(np.asarray(Wk[:, hs, :], np.float32).reshape(DMODEL, -1)),
            "wv": np.ascontiguousarray(np.asarray(Wv[:, hs, :], np.float32).reshape(DMODEL, -1)),
            "wd": np.ascontiguousarray(np.asarray(Wd[hs], np.float32).reshape(-1, DMODEL)),
            "bq": np.ascontiguousarray(np.asarray(bq[hs], np.float32).reshape(-1)),
            "bk": np.ascontiguousarray(np.asarray(bk[hs], np.float32).reshape(-1)),
            "bv": np.ascontiguousarray(np.asarray(bv[hs], np.float32).reshape(-1)),
        })
    return in_maps


def gather_outputs(results, bd):
    """Sum partial outputs per batch and add bd."""
    out = np.zeros((B, L, DMODEL), np.float32)
    per_b = NCORES // B
    for c, res in enumerate(results):
        out[c // per_b] += res["y"]
    out += np.asarray(bd, np.float32)[None, None, :]
    return out


def kernel(x, Wq, bq, Wk, bk, Wv, bv, Wd, bd, _trace=False):
    nc = _get_nc()
    in_maps = shard_inputs(x, Wq, bq, Wk, bk, Wv, bv, Wd, bd)
    res = run_bass_kernel_spmd(nc, in_maps, list(range(NCORES)), trace=_trace)
    out = gather_outputs(res.results, bd)
    if _trace:
        kernel.last_results = res
    return out


# revision 29
# speedup vs baseline: 1.0249x; 1.0249x over previous
"""Trainium2 Bass kernel: multi-head attention (dense transformer block).

Computation (per batch b):
    Q = x @ Wq + bq ; K = x @ Wk + bk ; V = x @ Wv + bv        (per head)
    P = exp((Q @ K^T) / sqrt(Dh))                               (no max-sub; scores are O(1))
    out = sum_h (P @ V / rowsum(P)) @ Wd[h] + bd

Sharding: 8 cores; core c handles batch b = c // 4 and 4 heads h0 = 4*(c%4).
Each core returns a partial [L, D] output; host sums groups of 4 cores + bd.

Per-core dataflow (SPMD program, all cores identical):
  - x^T built on-chip via PE transposes (f32 data, fp32r matmul path).
  - Heads processed in 2 pairs; QKV projections (fp32r) computed as Q^T/K^T
    [128 = 2 heads stacked on partitions, L], drained to bf16, so scores
    S^T = K^T.T @ Q^T (bf16, fast-weight-load) land with the softmax axis on
    PSUM partitions; exp on ScalarE (scale fused, 1024-wide) -> P^T bf16.
  - attend: O^T = [V_h | ones*64].T @ P^T — the 64 replicated ones columns
    make PSUM rows 64..127 the softmax denominator, already broadcast, for
    free; normalize = recip + mul on DVE, no cross-partition ops needed.
  - out-proj: Y = O^T.T @ Wd accumulated over head pairs in PSUM (fp32r).
"""

import os
import sys
from contextlib import ExitStack

import numpy as np

for _p in ("/opt/trn_rl_repo", "/root/.axon_site/_ro/trn_rl_repo"):
    if os.path.isdir(_p) and _p not in sys.path:
        sys.path.append(_p)

import concourse.bass as bass
import concourse.tile as tile
from concourse import bacc, mybir
from concourse.bass import ds, ts
from concourse.bass_utils import run_bass_kernel_spmd
from concourse.masks import make_identity
from concourse.tile_rust import add_dep_helper

F32 = mybir.dt.float32
F32R = mybir.dt.float32r
BF16 = mybir.dt.bfloat16

# Problem sizes (hardcoded per contract).
DMODEL, HEADS, DHEAD = 1024, 16, 64
B, L = 2, 2048
NCORES = 8
H_PER_CORE = B * HEADS // NCORES          # 4 heads per core
NPAIR = H_PER_CORE // 2                   # head pairs per core
P = 128                                   # partitions
KT = DMODEL // P                          # 8 k-tiles over dmodel
NLT = L // P                              # 16 l-tiles
LCH = 512                                 # matmul free-dim chunk (one psum bank)
ECH = 1024                                # exp chunk (2 psum banks)
NEC = L // ECH                            # 2 exp chunks
MCH = 512                                 # m-chunk for out-proj
NMC = DMODEL // MCH

BLOCK_DEPS = os.environ.get("ATT_BLOCK_DEPS", "1") == "1"


def build_nc():
    """Build the SPMD Bass program for one core."""
    nc = bacc.Bacc("TRN2", target_bir_lowering=False, debug=False,
                   num_devices=NCORES)

    x_d = nc.dram_tensor("x", [L, DMODEL], F32, kind="ExternalInput").ap()
    wq_d = nc.dram_tensor("wq", [DMODEL, H_PER_CORE * DHEAD], F32, kind="ExternalInput").ap()
    wk_d = nc.dram_tensor("wk", [DMODEL, H_PER_CORE * DHEAD], F32, kind="ExternalInput").ap()
    wv_d = nc.dram_tensor("wv", [DMODEL, H_PER_CORE * DHEAD], F32, kind="ExternalInput").ap()
    wd_d = nc.dram_tensor("wd", [H_PER_CORE * DHEAD, DMODEL], F32, kind="ExternalInput").ap()
    bq_d = nc.dram_tensor("bq", [H_PER_CORE * DHEAD], F32, kind="ExternalInput").ap()
    bk_d = nc.dram_tensor("bk", [H_PER_CORE * DHEAD], F32, kind="ExternalInput").ap()
    bv_d = nc.dram_tensor("bv", [H_PER_CORE * DHEAD], F32, kind="ExternalInput").ap()
    y_d = nc.dram_tensor("y", [L, DMODEL], F32, kind="ExternalOutput").ap()

    with ExitStack() as ctx:
        tc = ctx.enter_context(tile.TileContext(nc))
        _body(nc, tc, ctx, x_d, wq_d, wk_d, wv_d, wd_d, bq_d, bk_d, bv_d, y_d)
    nc.compile()
    return nc


def _body(nc, tc, ctx, x_d, wq_d, wk_d, wv_d, wd_d, bq_d, bk_d, bv_d, y_d):
    const = ctx.enter_context(tc.tile_pool(name="const", bufs=1))
    sb = ctx.enter_context(tc.tile_pool(name="sb", bufs=1))
    psum = ctx.enter_context(tc.tile_pool(name="psum", bufs=1, space="PSUM"))

    ident = const.tile([P, P], F32)
    make_identity(nc, ident)

    # biases: [pair*128 + i] layout matches head-pair partition stacking.
    bias_sb = const.tile([P, 3, NPAIR], F32)
    for i, b_d in enumerate((bq_d, bk_d, bv_d)):
        for p in range(NPAIR):
            nc.sync.dma_start(bias_sb[:, i, p:p + 1],
                              b_d.rearrange("(a p) -> a p", p=P)[p:p + 1, :]
                              .rearrange("a p -> p a"))
    # bv replicated across partitions (free-axis bias for the V drain)
    bv_rep = const.tile([P, NPAIR * P], F32)
    nc.sync.dma_start(bv_rep, bass.AP(tensor=bv_d.tensor, offset=0,
                                      ap=[[0, P], [1, NPAIR * P]]))

    # shared [128,1024] psum slots for scores & transposes
    def sc_tile(shape=None, dt=F32):
        return psum.tile(shape or [P, ECH], dt, tag="sctr", bufs=3, name="sctr")

    # qkv weights for all pairs first (so QKV can start asap), bf16 on device
    w_sb = const.tile([P, NPAIR, 3, KT, P], BF16)
    for p in range(NPAIR):
        ws = sb.tile([P, 3, KT, P], F32, tag="wstage", bufs=1)
        for i, w_d in enumerate((wq_d, wk_d, wv_d)):
            nc.sync.dma_start(
                ws[:, i],
                w_d.rearrange("(kt k) m -> k kt m", k=P)[:, :, ds(p * P, P)])
        nc.vector.tensor_copy(w_sb[:, p], ws)

    # out-proj weights, bf16 on device
    wd_sb = const.tile([P, NPAIR, DMODEL], BF16)
    wds = sb.tile([P, NPAIR, DMODEL], F32, tag="wstage2", bufs=1)
    nc.sync.dma_start(wds, wd_d.rearrange("(pp k) m -> k pp m", k=P))
    nc.vector.tensor_copy(wd_sb, wds)

    # ---- phase 0: x^T bf16 via PE transposes (f32 in, bf16 out drain) ----
    xt = sb.tile([P, KT, L], BF16)
    for lt in range(NLT):
        xs = sb.tile([P, DMODEL], F32, tag="xstage", bufs=4)
        eng = nc.sync if lt % 2 == 0 else nc.gpsimd
        eng.dma_start(xs, x_d[ds(lt * P, P), :])
        for kt in range(KT):
            tp = sc_tile([P, P], F32)
            nc.tensor.transpose(tp, xs[:, ds(kt * P, P)], ident)
            nc.vector.tensor_copy(xt[:, kt, ds(lt * P, P)], tp)

    o_norm = sb.tile([P, NPAIR, L], BF16)

    # ---- per pair: K -> Q(chunk0) -> V -> blocks(chunk0) -> Q(chunk1)... ----
    def qkv_proj(dst, p, i, lcs):
        for lc in lcs:
            ps = psum.tile([P, LCH], F32, tag="qkvp", bufs=2, name="qkvps")
            for kt in range(KT):
                nc.tensor.matmul(
                    ps, lhsT=w_sb[:, p, i, kt],
                    rhs=xt[:, kt, ds(lc * LCH, LCH)],
                    start=(kt == 0), stop=(kt == KT - 1))
            nc.vector.tensor_scalar_add(
                dst[:, ds(lc * LCH, LCH)], ps, bias_sb[:, i, p:p + 1])

    for p in range(NPAIR):
        qT = sb.tile([P, L], BF16, tag="qkv0", bufs=NPAIR)
        kT_sb = sb.tile([P, L], BF16, tag="qkv1", bufs=NPAIR)
        qkv_proj(kT_sb, p, 1, range(L // LCH))
        qkv_proj(qT, p, 0, range(ECH // LCH))

        # V computed directly in [l', d] layout: per head [V_h (64) | ones]
        vt = sb.tile([P, NLT, 2 * P], BF16, tag="vt", bufs=NPAIR)
        nc.vector.memset(vt[:, :, DHEAD:P], 1.0)
        nc.vector.memset(vt[:, :, P + DHEAD:2 * P], 1.0)
        for lt in range(NLT):
            vp = psum.tile([P, P], F32, tag="qkvp", bufs=2, name="vp")
            for kt in range(KT):
                nc.tensor.matmul(
                    vp, lhsT=xt[:, kt, ds(lt * P, P)],
                    rhs=w_sb[:, p, 2, kt],
                    start=(kt == 0), stop=(kt == KT - 1))
            nc.vector.tensor_add(vt[:, lt, 0:DHEAD], vp[:, 0:DHEAD],
                                 bv_rep[:, ds(p * P, DHEAD)])
            nc.vector.tensor_add(vt[:, lt, P:P + DHEAD], vp[:, DHEAD:P],
                                 bv_rep[:, ds(p * P + DHEAD, DHEAD)])

        # blocks: h-interleaved scores (dual 64-row PE tiles) -> exp -> attend
        for ec in range(NEC):
            if ec > 0:
                qkv_proj(qT, p, 0, range(ec * ECH // LCH, (ec + 1) * ECH // LCH))
            pt_tiles = [[None] * NLT, [None] * NLT]
            for lt in range(NLT):
                for h in range(2):
                    sp = sc_tile()
                    for sub in range(ECH // LCH):
                        nc.tensor.matmul(
                            sp[:, ds(sub * LCH, LCH)],
                            lhsT=kT_sb[ds(64 * h, 64), ds(lt * P, P)],
                            rhs=qT[ds(64 * h, 64),
                                   ds(ec * ECH + sub * LCH, LCH)],
                            start=True, stop=True)
                    pt = sb.tile([P, ECH], BF16, tag="pt", bufs=36)
                    nc.scalar.activation(
                        pt, sp, func=mybir.ActivationFunctionType.Exp,
                        scale=1.0 / np.sqrt(DHEAD))
                    pt_tiles[h][lt] = pt

            for sub in range(ECH // LCH):
                lc = ec * ECH + sub * LCH
                for h in range(2):
                    op = psum.tile([P, LCH], F32, tag="qkvp", bufs=2, name="op")
                    for lt in range(NLT):
                        nc.tensor.matmul(
                            op, lhsT=vt[:, lt, ds(P * h, P)],
                            rhs=pt_tiles[h][lt][:, ds(sub * LCH, LCH)],
                            start=(lt == 0), stop=(lt == NLT - 1))
                    # rows 64..127 are the denominator, already broadcast
                    rs = sb.tile([DHEAD, LCH], F32, tag="rs", bufs=2)
                    nc.vector.reciprocal(rs, op[DHEAD:P, :])
                    nc.vector.tensor_mul(
                        o_norm[ds(64 * h, 64), p, ds(lc, LCH)],
                        op[0:DHEAD, :], rs)

            # out-projection for this chunk once the last pair finished it
            if p == NPAIR - 1:
                for lt in range(ec * NLT // NEC, (ec + 1) * NLT // NEC):
                    for mc in range(NMC):
                        yp = psum.tile([P, MCH], F32, tag="qkvp", bufs=2)
                        for pp in range(NPAIR):
                            nc.tensor.matmul(
                                yp, lhsT=o_norm[:, pp, ds(lt * P, P)],
                                rhs=wd_sb[:, pp, ds(mc * MCH, MCH)],
                                start=(pp == 0), stop=(pp == NPAIR - 1))
                        ys = sb.tile([P, MCH], F32, tag="ys", bufs=3)
                        nc.vector.tensor_copy(ys, yp)
                        nc.sync.dma_start(
                            y_d[ds(lt * P, P), ds(mc * MCH, MCH)], ys)


_NC_CACHE = {}


def _get_nc():
    if "nc" not in _NC_CACHE:
        _NC_CACHE["nc"] = build_nc()
    return _NC_CACHE["nc"]


def shard_inputs(x, Wq, bq, Wk, bk, Wv, bv, Wd, bd):
    """Build the 8 per-core input maps."""
    in_maps = []
    for c in range(NCORES):
        b = c // (NCORES // B)
        h0 = (c % (NCORES // B)) * H_PER_CORE
        hs = slice(h0, h0 + H_PER_CORE)
        in_maps.append({
            "x": np.ascontiguousarray(np.asarray(x[b], np.float32)),
            "wq": np.ascontiguousarray(np.asarray(Wq[:, hs, :], np.float32).reshape(DMODEL, -1)),
            "wk": np.ascontiguousarray(np.asarray(Wk[:, hs, :], np.float32).reshape(DMODEL, -1)),
            "wv": np.ascontiguousarray(np.asarray(Wv[:, hs, :], np.float32).reshape(DMODEL, -1)),
            "wd": np.ascontiguousarray(np.asarray(Wd[hs], np.float32).reshape(-1, DMODEL)),
            "bq": np.ascontiguousarray(np.asarray(bq[hs], np.float32).reshape(-1)),
            "bk": np.ascontiguousarray(np.asarray(bk[hs], np.float32).reshape(-1)),
            "bv": np.ascontiguousarray(np.asarray(bv[hs], np.float32).reshape(-1)),
        })
    return in_maps


def gather_outputs(results, bd):
    """Sum partial outputs per batch and add bd."""
    out = np.zeros((B, L, DMODEL), np.float32)
    per_b = NCORES // B
    for c, res in enumerate(results):
        out[c // per_b] += res["y"]
    out += np.asarray(bd, np.float32)[None, None, :]
    return out


def kernel(x, Wq, bq, Wk, bk, Wv, bv, Wd, bd, _trace=False):
    nc = _get_nc()
    in_maps = shard_inputs(x, Wq, bq, Wk, bk, Wv, bv, Wd, bd)
    res = run_bass_kernel_spmd(nc, in_maps, list(range(NCORES)), trace=_trace)
    out = gather_outputs(res.results, bd)
    if _trace:
        kernel.last_results = res
    return out
